# revision 1
# baseline (speedup 1.0000x reference)
"""Trainium2 Bass kernel for CustomMultiHeadAttention.

Problem: B=2, S=2048, D=2048, H=16 heads of Hd=128; y = MHA(q, k, v) with
torch-Linear-style projections (x @ W.T + b) and out projection.

Sharding (8 NeuronCores): data-parallel over batch (2 groups) x tensor-
parallel over heads (4 heads = 512 channels per core). Each core computes
its 4 heads' Q/K/V projections, attention, and a column-sharded partial of
the out projection; the host sums the 4 partials per batch and adds bo.

Per-core device program (all matmuls bf16, K=128 contraction, N=512):
  - host passes x.T and W.T contiguous so no transposes are needed on-chip
  - Q^T, K^T computed channels-major [512c, S]; V sequence-major [S, 512e]
  - scores computed transposed: S^T[t,s] = K^T(:,t)^T Q^T(:,s); exp on ACT
    (scale 1/sqrt(128) folded in) over two-bank PSUM tiles
  - softmax denominator l[s] accumulated on the PE itself: a ones-column
    matmul per exp tile into a [1,512] PSUM row, interleaved with the
    P~V accumulation (keeps the PE stream dense; nothing on the PE waits
    for DVE chains)
  - 1/l applied via GPSIMD partition_broadcast + DVE reciprocal/mul; V
    bias folded in after normalization (sum_t (V+bv) P~ = P~V + bv*l).
  - partial out-proj Z^T[d,s] = Wo_loc^T O^T accumulated over 512 local
    channels, emitted bf16.
"""

import math

import numpy as np
import ml_dtypes

B = 2
S = 2048
D = 2048
HD = 128          # head dim
TP = 4            # head-group (tensor-parallel) factor
CL = D // TP      # 512 local channels = 4 heads per core
NCORES = 8

_NC = None


def _build_nc(s=S, d=D, cl=CL):
    """Build the per-core Bass program (SPMD: same program, 8 cores)."""
    from contextlib import ExitStack

    import concourse.bass as bass
    import concourse.mybir as mybir
    import concourse.tile as tile
    from concourse import bacc, bass_isa

    f32 = mybir.dt.float32
    bf16 = mybir.dt.bfloat16
    Exp = mybir.ActivationFunctionType.Exp

    SBW = 512                 # matmul moving free dim / PSUM bank width
    W2 = 2 * SBW              # two-bank psum tile width
    NSB = s // SBW            # s blocks
    NTB = s // 128            # t blocks (key/value seq chunks)
    KC = d // 128             # contraction chunks over model dim
    NH = cl // HD             # local heads
    SCALE = 1.0 / math.sqrt(HD)

    nc = bacc.Bacc("TRN2", target_bir_lowering=False, debug=False)

    xqT = nc.dram_tensor("xqT", [d, s], bf16, kind="ExternalInput").ap()
    xkT = nc.dram_tensor("xkT", [d, s], bf16, kind="ExternalInput").ap()
    xvT = nc.dram_tensor("xvT", [d, s], bf16, kind="ExternalInput").ap()
    wqT = nc.dram_tensor("wqT", [d, cl], bf16, kind="ExternalInput").ap()
    wkT = nc.dram_tensor("wkT", [d, cl], bf16, kind="ExternalInput").ap()
    wvT = nc.dram_tensor("wvT", [d, cl], bf16, kind="ExternalInput").ap()
    woT = nc.dram_tensor("woT", [cl, d], bf16, kind="ExternalInput").ap()
    bq = nc.dram_tensor("bq", [cl], f32, kind="ExternalInput").ap()
    bk = nc.dram_tensor("bk", [cl], f32, kind="ExternalInput").ap()
    bv = nc.dram_tensor("bv", [cl], f32, kind="ExternalInput").ap()
    zT = nc.dram_tensor("zT", [d, s], bf16, kind="ExternalOutput").ap()

    with tile.TileContext(nc) as tc, ExitStack() as ctx:
        const = ctx.enter_context(tc.tile_pool(name="const", bufs=1))
        qkv = ctx.enter_context(tc.tile_pool(name="qkv", bufs=1))
        wop = ctx.enter_context(tc.tile_pool(name="wop", bufs=1))
        # PSUM: mm pool tiles are 2 banks wide; 2*2 + ot 2 + l 2 = 8 banks.
        ps_mm = ctx.enter_context(tc.tile_pool(name="ps_mm", bufs=2, space="PSUM"))
        ps_ot = ctx.enter_context(tc.tile_pool(name="ps_ot", bufs=2, space="PSUM"))
        ps_l = ctx.enter_context(tc.tile_pool(name="ps_l", bufs=2, space="PSUM"))

        ones_col = const.tile([128, 1], bf16, tag="ones_col")
        nc.vector.memset(ones_col, 1.0)

        # Biases land via DMA into *_dma, then hop to *_sb on the DVE so
        # downstream tensor_scalar ops only carry their PE wait.
        def load_bias(b_dram, nm):
            b_dma = const.tile([128, NH], f32, tag=f"{nm}d", name=f"{nm}d")
            nc.sync.dma_start(b_dma, b_dram.rearrange("(m p) -> p m", p=128))
            b_sb = const.tile([128, NH], f32, tag=nm, name=nm)
            nc.vector.tensor_copy(b_sb, b_dma)
            return b_sb

        bq_sb = load_bias(bq, "bq")
        bk_sb = load_bias(bk, "bk")
        bv_sb = load_bias(bv, "bv")

        wo_sb = wop.tile([128, NH, d], bf16, tag="wo")
        nc.sync.dma_start(wo_sb, woT.rearrange("(k p) m -> p k m", p=128))

        # persistent activation tiles
        qt = [qkv.tile([128, s], bf16, tag=f"qt{h}", name=f"qt{h}") for h in range(NH)]
        kt = [qkv.tile([128, s], bf16, tag=f"kt{h}", name=f"kt{h}") for h in range(NH)]
        vt = [qkv.tile([128, cl], bf16, tag=f"vt{t}", name=f"vt{t}") for t in range(NTB)]
        ot = [qkv.tile([128, s], bf16, tag=f"ot{h}", name=f"ot{h}") for h in range(NH)]

        # ---- Phase A: projections -------------------------------------
        with tc.tile_pool(name="wqkv", bufs=1) as wp, \
             tc.tile_pool(name="panels", bufs=2) as panels:

            wq_sb = wp.tile([128, KC, cl], bf16, tag="wq")
            nc.sync.dma_start(wq_sb, wqT.rearrange("(k p) m -> p k m", p=128))
            wk_sb = wp.tile([128, KC, cl], bf16, tag="wk")
            nc.sync.dma_start(wk_sb, wkT.rearrange("(k p) m -> p k m", p=128))
            wv_sb = wp.tile([128, KC, cl], bf16, tag="wv")
            nc.sync.dma_start(wv_sb, wvT.rearrange("(k p) m -> p k m", p=128))

            def qk_proj(x_dram, w_sb, b_sb, outs):
                # outs[m][c, s] = sum_d W[m*128+c, d] x[s, d]  (+ bias)
                # two n-blocks share one 2-bank psum tile -> one wide drain
                for np_ in range(NSB // 2):
                    xp = panels.tile([128, KC, W2], bf16, tag="xpanel",
                                     name=f"xp{np_}")
                    nc.sync.dma_start(
                        xp, x_dram[:, np_ * W2:(np_ + 1) * W2]
                        .rearrange("(k p) s -> p k s", p=128))
                    for m in range(NH):
                        ps = ps_mm.tile([128, W2], f32, tag="mm", name="ps_proj")
                        for k in range(KC):
                            w_ap = w_sb[:, k, m * 128:(m + 1) * 128]
                            nc.tensor.matmul(
                                ps[:, 0:SBW], lhsT=w_ap, rhs=xp[:, k, 0:SBW],
                                start=(k == 0), stop=(k == KC - 1))
                            nc.tensor.matmul(
                                ps[:, SBW:W2], lhsT=w_ap, rhs=xp[:, k, SBW:W2],
                                start=(k == 0), stop=(k == KC - 1))
                        nc.vector.tensor_scalar_add(
                            outs[m][:, np_ * W2:(np_ + 1) * W2], ps,
                            b_sb[:, m:m + 1])

            qk_proj(xqT, wq_sb, bq_sb, qt)
            qk_proj(xkT, wk_sb, bk_sb, kt)

            # V sequence-major: vt[t][tt, e] = sum_d x[t*128+tt, d] Wv[e, d]
            for n in range(NSB):
                xp = panels.tile([128, KC, SBW], bf16, tag="xpanel",
                                 name=f"xpv{n}")
                nc.sync.dma_start(
                    xp, xvT[:, n * SBW:(n + 1) * SBW]
                    .rearrange("(k p) s -> p k s", p=128))
                for tsub in range(SBW // 128):
                    t = n * (SBW // 128) + tsub
                    ps = ps_mm.tile([128, cl], f32, tag="mm", name="ps_v")
                    for k in range(KC):
                        nc.tensor.matmul(
                            ps, lhsT=xp[:, k, tsub * 128:(tsub + 1) * 128],
                            rhs=wv_sb[:, k, :],
                            start=(k == 0), stop=(k == KC - 1))
                    nc.vector.tensor_copy(vt[t], ps)

        # ---- Phase B: attention per (head, s-block) --------------------
        with tc.tile_pool(name="ptiles", bufs=2 * (NTB // 2)) as ppool, \
             tc.tile_pool(name="small", bufs=2) as small:

            for h in range(NH):
                for sb in range(NSB):
                    ssl = slice(sb * SBW, (sb + 1) * SBW)
                    ops_ = ps_ot.tile([128, SBW], f32, tag="ot", name="ps_pv")
                    lps = ps_l.tile([1, SBW], f32, tag="l", name="ps_l")
                    for tp2 in range(NTB // 2):
                        ps = ps_mm.tile([128, W2], f32, tag="mm", name="ps_sc")
                        for half in range(2):
                            tb = tp2 * 2 + half
                            nc.tensor.matmul(
                                ps[:, half * SBW:(half + 1) * SBW],
                                lhsT=kt[h][:, tb * 128:(tb + 1) * 128],
                                rhs=qt[h][:, ssl], start=True, stop=True)
                        p2 = ppool.tile([128, W2], bf16, tag="p", name="ptile")
                        nc.scalar.activation(p2, ps, Exp, scale=SCALE)
                        for half in range(2):
                            tb = tp2 * 2 + half
                            psl = p2[:, half * SBW:(half + 1) * SBW]
                            nc.tensor.matmul(
                                ops_, lhsT=vt[tb][:, h * 128:(h + 1) * 128],
                                rhs=psl,
                                start=(tb == 0), stop=(tb == NTB - 1),
                                skip_group_check=True)
                            nc.tensor.matmul(
                                lps, lhsT=ones_col, rhs=psl,
                                start=(tb == 0), stop=(tb == NTB - 1),
                                skip_group_check=True)

                    # 1/l path stays off the PE: DVE copy -> GPSIMD bcast
                    # -> DVE full-lane reciprocal -> DVE mul (+ bv).
                    l_sb = small.tile([1, SBW], f32, tag="l_sb", name="l_sb")
                    nc.vector.tensor_copy(l_sb, lps)
                    lb = small.tile([128, SBW], f32, tag="lb", name="lb")
                    nc.gpsimd.partition_broadcast(lb, l_sb)
                    rb_sb = small.tile([128, SBW], f32, tag="rb", name="rb_sb")
                    nc.vector.reciprocal(rb_sb, lb)
                    osl = ot[h][:, ssl]
                    nc.vector.tensor_mul(osl, ops_, rb_sb)
                    nc.vector.tensor_scalar_add(osl, osl, bv_sb[:, h:h + 1])

        # ---- Phase C: partial out-projection ---------------------------
        with tc.tile_pool(name="zout", bufs=3) as zpool:
            for dd in range(KC):
                for sp2 in range(NSB // 2):
                    ps = ps_mm.tile([128, W2], f32, tag="mm", name="ps_z")
                    for eb in range(NH):
                        wo_ap = wo_sb[:, eb, dd * 128:(dd + 1) * 128]
                        for half in range(2):
                            sb = sp2 * 2 + half
                            nc.tensor.matmul(
                                ps[:, half * SBW:(half + 1) * SBW],
                                lhsT=wo_ap,
                                rhs=ot[eb][:, sb * SBW:(sb + 1) * SBW],
                                start=(eb == 0), stop=(eb == NH - 1),
                                skip_group_check=True)
                    zt = zpool.tile([128, W2], bf16, tag="z", name="z_sb")
                    nc.vector.tensor_copy(zt, ps)
                    nc.sync.dma_start(
                        zT[dd * 128:(dd + 1) * 128, sp2 * W2:(sp2 + 1) * W2],
                        zt)

    nc.compile()
    return nc


def _bf16(a):
    return np.ascontiguousarray(a).astype(ml_dtypes.bfloat16)


def _in_maps(inputs):
    q = np.asarray(inputs["query"], dtype=np.float32)
    k = np.asarray(inputs["key_in"], dtype=np.float32)
    v = np.asarray(inputs["value"], dtype=np.float32)
    Wq = np.asarray(inputs["Wq"], dtype=np.float32)
    Wk = np.asarray(inputs["Wk"], dtype=np.float32)
    Wv = np.asarray(inputs["Wv"], dtype=np.float32)
    Wo = np.asarray(inputs["Wo"], dtype=np.float32)
    bq = np.asarray(inputs["bq"], dtype=np.float32)
    bk = np.asarray(inputs["bk"], dtype=np.float32)
    bv = np.asarray(inputs["bv"], dtype=np.float32)

    xT = [[_bf16(x[b].T) for b in range(B)] for x in (q, k, v)]
    maps = []
    for c in range(NCORES):
        b, g = divmod(c, TP)
        sl = slice(g * CL, (g + 1) * CL)
        maps.append({
            "xqT": xT[0][b], "xkT": xT[1][b], "xvT": xT[2][b],
            "wqT": _bf16(Wq[sl, :].T), "wkT": _bf16(Wk[sl, :].T),
            "wvT": _bf16(Wv[sl, :].T), "woT": _bf16(Wo[:, sl].T),
            "bq": np.ascontiguousarray(bq[sl]),
            "bk": np.ascontiguousarray(bk[sl]),
            "bv": np.ascontiguousarray(bv[sl]),
        })
    return maps


TRACE = False
TMPDIR = None
LAST_RESULT = None


def kernel(**inputs):
    global _NC, LAST_RESULT
    from concourse.bass_utils import run_bass_kernel_spmd

    if _NC is None:
        _NC = _build_nc()
    maps = _in_maps(inputs)
    res = run_bass_kernel_spmd(_NC, maps, core_ids=list(range(NCORES)),
                               trace=TRACE, tmpdir=TMPDIR)
    LAST_RESULT = res

    bo = np.asarray(inputs["bo"], dtype=np.float32)
    out = np.zeros((B, S, D), dtype=np.float32)
    for c in range(NCORES):
        b, _ = divmod(c, TP)
        out[b] += res.results[c]["zT"].astype(np.float32).T
    out += bo[None, None, :]
    return out



# revision 11
# speedup vs baseline: 1.2801x; 1.2801x over previous
"""Trainium2 Bass kernel for CustomMultiHeadAttention.

Problem: B=2, S=2048, D=2048, H=16 heads of Hd=128; y = MHA(q, k, v) with
torch-Linear-style projections (x @ W.T + b) and out projection.

Sharding (8 NeuronCores): data-parallel over batch (2 groups) x tensor-
parallel over heads (4 heads = 512 channels per core). Each core computes
its 4 heads' Q/K/V projections, attention, and a column-sharded partial of
the out projection; the host sums the 4 partials per batch and adds
bo + Wo @ bv (bv commutes with the out projection, so it never goes
on-device).

Per-core device program (all matmuls bf16, K=128 contraction):
  Phase A (projections): Q^T/K^T channels-major [512c, S]; V sequence-major
    [S, 512e]. DMA order puts wq + the first x panel first so the PE starts
    ~15us in; wk/wv stream behind, wo is deferred to phase B.
  Phase B+C (attention + out-proj, software-pipelined): per (h, s-block)
    iteration the PE emits scores for iter i interleaved (per 2-t-block
    slot) with PV accumulation for iter i-1 and a trickle of out-proj
    matmuls for s-block sb-1.  Softmax denominators come from a DVE add
    chain over the 8 exp tiles + ONE ones-column matmul (512 cols) instead
    of 16 -- the PE streams P only once for PV.  1/l via
    reciprocal_approx_fast on the [1,512] row, then GPSIMD broadcast and
    one DVE mul.  Out-proj partials drain via DVE copy + DMA as they
    complete; only the last s-block's out-proj runs as a tail.
"""

import math

import numpy as np
import ml_dtypes

B = 2
S = 2048
D = 2048
HD = 128          # head dim
TP = 4            # head-group (tensor-parallel) factor
CL = D // TP      # 512 local channels = 4 heads per core
NCORES = 8

_NC = None


def _build_nc(s=S, d=D, cl=CL):
    """Build the per-core Bass program (SPMD: same program, 8 cores)."""
    from contextlib import ExitStack

    import concourse.bass as bass
    import concourse.mybir as mybir
    import concourse.tile as tile
    from concourse import bacc, bass_isa

    f32 = mybir.dt.float32
    bf16 = mybir.dt.bfloat16
    Exp = mybir.ActivationFunctionType.Exp

    SBW = 512                 # matmul moving free dim / PSUM bank width
    W2 = 2 * SBW              # two-bank psum tile width
    NSB = s // SBW            # s blocks
    NTB = s // 128            # t blocks (key/value seq chunks)
    NT2 = NTB // 2            # tp2 slots per (h, sb)
    KC = d // 128             # contraction chunks over model dim
    NH = cl // HD             # local heads
    SCALE = 1.0 / math.sqrt(HD)
    C_PER_SLOT = 3            # out-proj matmuls trickled per tp2 slot

    nc = bacc.Bacc("TRN2", target_bir_lowering=False, debug=False)

    xqT = nc.dram_tensor("xqT", [d, s], bf16, kind="ExternalInput").ap()
    xkT = nc.dram_tensor("xkT", [d, s], bf16, kind="ExternalInput").ap()
    xvT = nc.dram_tensor("xvT", [d, s], bf16, kind="ExternalInput").ap()
    wqT = nc.dram_tensor("wqT", [d, cl], bf16, kind="ExternalInput").ap()
    wkT = nc.dram_tensor("wkT", [d, cl], bf16, kind="ExternalInput").ap()
    wvT = nc.dram_tensor("wvT", [d, cl], bf16, kind="ExternalInput").ap()
    woT = nc.dram_tensor("woT", [cl, d], bf16, kind="ExternalInput").ap()
    bq = nc.dram_tensor("bq", [cl], f32, kind="ExternalInput").ap()
    bk = nc.dram_tensor("bk", [cl], f32, kind="ExternalInput").ap()
    zT = nc.dram_tensor("zT", [d, s], bf16, kind="ExternalOutput").ap()

    with tile.TileContext(nc) as tc, ExitStack() as ctx:
        const = ctx.enter_context(tc.tile_pool(name="const", bufs=1))
        qkv = ctx.enter_context(tc.tile_pool(name="qkv", bufs=1))
        wop = ctx.enter_context(tc.tile_pool(name="wop", bufs=1))
        # PSUM budget (8 banks, pools are 2-bank granular): sc 2x2 + one
        # 2-bank tile whose halves ping-pong as PV accumulators + one
        # 2-bank tile split into out-proj group bank | l-row bank.
        ps_mm = ctx.enter_context(tc.tile_pool(name="ps_mm", bufs=2, space="PSUM"))
        ps_ot = ctx.enter_context(tc.tile_pool(name="ps_ot", bufs=1, space="PSUM"))
        ps_zl = ctx.enter_context(tc.tile_pool(name="ps_zl", bufs=1, space="PSUM"))

        ones_col = const.tile([128, 1], bf16, tag="ones_col")
        nc.vector.memset(ones_col, 1.0)

        # Biases land via DMA into *_dma, then hop to *_sb on the DVE so
        # downstream tensor_scalar ops only carry their PE wait.
        def load_bias(b_dram, nm):
            b_dma = const.tile([128, NH], f32, tag=f"{nm}d", name=f"{nm}d")
            nc.sync.dma_start(b_dma, b_dram.rearrange("(m p) -> p m", p=128))
            b_sb = const.tile([128, NH], f32, tag=nm, name=nm)
            nc.vector.tensor_copy(b_sb, b_dma)
            return b_sb

        bq_sb = load_bias(bq, "bq")
        bk_sb = load_bias(bk, "bk")

        # wo tile allocated now, DMA deferred until phase B (saves startup
        # DMA bandwidth for wq + the first x panel).
        wo_sb = wop.tile([128, NH, d], bf16, tag="wo")

        # persistent activation tiles
        qt = [qkv.tile([128, s], bf16, tag=f"qt{h}", name=f"qt{h}") for h in range(NH)]
        kt = [qkv.tile([128, s], bf16, tag=f"kt{h}", name=f"kt{h}") for h in range(NH)]
        vt = [qkv.tile([128, cl], bf16, tag=f"vt{t}", name=f"vt{t}") for t in range(NTB)]
        ot = [qkv.tile([128, s], bf16, tag=f"ot{h}", name=f"ot{h}") for h in range(NH)]

        # ---- Phase A: projections -------------------------------------
        with tc.tile_pool(name="wqkv", bufs=1) as wp, \
             tc.tile_pool(name="panels", bufs=2) as panels:

            wq_sb = wp.tile([128, KC, cl], bf16, tag="wq")
            wq_re = wqT.rearrange("(k p) m -> p k m", p=128)
            # split across two queues so the gating transfer finishes sooner
            nc.sync.dma_start(wq_sb[:, 0:KC // 2], wq_re[:, 0:KC // 2])
            nc.sync.dma_start(wq_sb[:, KC // 2:KC], wq_re[:, KC // 2:KC])
            wk_sb = wp.tile([128, KC, cl], bf16, tag="wk")
            wv_sb = wp.tile([128, KC, cl], bf16, tag="wv")

            def load_panel(x_dram, lo, hi, nm, split):
                xp = panels.tile([128, KC, hi - lo], bf16, tag="xpanel", name=nm)
                xr = x_dram[:, lo:hi].rearrange("(k p) s -> p k s", p=128)
                if split:
                    nc.sync.dma_start(xp[:, 0:KC // 2], xr[:, 0:KC // 2])
                    nc.sync.dma_start(xp[:, KC // 2:KC], xr[:, KC // 2:KC])
                else:
                    nc.sync.dma_start(xp, xr)
                return xp

            def qk_proj(x_dram, w_sb, b_sb, outs, hook=None):
                # outs[m][c, s] = sum_d W[m*128+c, d] x[s, d]  (+ bias)
                for np_ in range(NSB // 2):
                    xp = load_panel(x_dram, np_ * W2, (np_ + 1) * W2,
                                    f"xp{np_}", split=(np_ == 0 and hook))
                    if np_ == 0 and hook:
                        hook()
                    for m in range(NH):
                        ps = ps_mm.tile([128, W2], f32, tag="mm", name="ps_proj")
                        for k in range(KC):
                            w_ap = w_sb[:, k, m * 128:(m + 1) * 128]
                            nc.tensor.matmul(
                                ps[:, 0:SBW], lhsT=w_ap, rhs=xp[:, k, 0:SBW],
                                start=(k == 0), stop=(k == KC - 1))
                            nc.tensor.matmul(
                                ps[:, SBW:W2], lhsT=w_ap, rhs=xp[:, k, SBW:W2],
                                start=(k == 0), stop=(k == KC - 1))
                        nc.vector.tensor_scalar_add(
                            outs[m][:, np_ * W2:(np_ + 1) * W2], ps,
                            b_sb[:, m:m + 1])

            def issue_wk_wv():
                nc.sync.dma_start(wk_sb, wkT.rearrange("(k p) m -> p k m", p=128))
                nc.sync.dma_start(wv_sb, wvT.rearrange("(k p) m -> p k m", p=128))

            qk_proj(xqT, wq_sb, bq_sb, qt, hook=issue_wk_wv)
            qk_proj(xkT, wk_sb, bk_sb, kt)

            # V sequence-major: vt[t][tt, e] = sum_d x[t*128+tt, d] Wv[e, d]
            for n in range(NSB):
                xp = load_panel(xvT, n * SBW, (n + 1) * SBW, f"xpv{n}", False)
                for tsub in range(SBW // 128):
                    t = n * (SBW // 128) + tsub
                    ps = ps_mm.tile([128, cl], f32, tag="mm", name="ps_v")
                    for k in range(KC):
                        nc.tensor.matmul(
                            ps, lhsT=xp[:, k, tsub * 128:(tsub + 1) * 128],
                            rhs=wv_sb[:, k, :],
                            start=(k == 0), stop=(k == KC - 1))
                    nc.vector.tensor_copy(vt[t], ps)

        # ---- Phase B+C: attention + out-proj, software-pipelined -------
        with tc.tile_pool(name="ptiles", bufs=2 * NT2) as ppool, \
             tc.tile_pool(name="accp", bufs=6) as accp, \
             tc.tile_pool(name="small", bufs=4) as small, \
             tc.tile_pool(name="zout", bufs=3) as zpool:

            nc.sync.dma_start(wo_sb, woT.rearrange("(k p) m -> p k m", p=128))

            # persistent psum: PV accumulator halves ping-pong per
            # iteration; z bank | l row share the other 2-bank tile
            ops2 = ps_ot.tile([128, W2], f32, tag="ops2", name="ops2")
            pz2 = ps_zl.tile([128, W2], f32, tag="pz2", name="pz2")
            z_ps = pz2[:, 0:SBW]
            lps = pz2[0:1, SBW:W2]

            # --- out-proj (phase C) work queue: per-matmul closures ---
            cqueue = []

            def make_c_work(sb, ps_get):
                ssl = slice(sb * SBW, (sb + 1) * SBW)
                box = {}

                def mk(dd, eb, getter):
                    def emit():
                        if eb == 0:
                            box["ps"] = getter()
                        nc.tensor.matmul(
                            box["ps"], lhsT=wo_sb[:, eb, dd * 128:(dd + 1) * 128],
                            rhs=ot[eb][:, ssl],
                            start=(eb == 0), stop=(eb == NH - 1),
                            skip_group_check=True)
                        if eb == NH - 1:
                            zt = zpool.tile([128, SBW], bf16, tag="z", name="z_sb")
                            nc.vector.tensor_copy(zt, box["ps"])
                            nc.sync.dma_start(
                                zT[dd * 128:(dd + 1) * 128, ssl], zt)
                    return emit

                return [mk(dd, eb, ps_get[dd % len(ps_get)])
                        for dd in range(KC) for eb in range(NH)]

            def drain_c(k):
                for _ in range(min(k, len(cqueue))):
                    cqueue.pop(0)()

            # --- per-iteration state for the software pipeline ---
            def finish_prev_tree(st):
                # DVE: sum the 8 exp tiles -> acc2 [128,SBW] bf16.  Adds are
                # either (bf16,bf16)->f32 or (f32,f32)->f32 -- never mixed.
                a = accp.tile([128, W2], f32, tag="acc", name="accA")
                b = accp.tile([128, W2], f32, tag="acc", name="accB")
                nc.vector.tensor_add(a, st["p2"][0], st["p2"][1])
                for j in range(1, NT2 // 2):
                    nc.vector.tensor_add(b, st["p2"][2 * j], st["p2"][2 * j + 1])
                    nc.vector.tensor_add(a, a, b)
                acc2 = accp.tile([128, SBW], bf16, tag="acc2", name="acc2")
                nc.vector.tensor_add(acc2, a[:, 0:SBW], a[:, SBW:W2])
                st["acc2"] = acc2

            def finish_prev_post(st):
                # PE: single ones-matmul for the softmax denominator row
                nc.tensor.matmul(lps, lhsT=ones_col, rhs=st["acc2"],
                                 start=True, stop=True, skip_group_check=True)
                # DVE: 1/l on the row, GPSIMD broadcast, DVE normalize
                rl = small.tile([1, SBW], f32, tag="rl", name="rl")
                nc.vector.reciprocal_approx_fast(out=rl, in_=lps)
                rb = small.tile([128, SBW], f32, tag="rb", name="rb")
                nc.gpsimd.partition_broadcast(rb, rl)
                h, ssl = st["h"], st["ssl"]
                nc.vector.tensor_mul(ot[h][:, ssl], st["ops"], rb)

            state = None
            iters = [(sb, h) for sb in range(NSB) for h in range(NH)]
            for sb, h in iters:
                ssl = slice(sb * SBW, (sb + 1) * SBW)
                prev = state
                if prev is not None:
                    par = prev["par"]
                    prev["ops"] = ops2[:, par * SBW:(par + 1) * SBW]
                cur = {"h": h, "sb": sb, "ssl": ssl, "p2": [],
                       "par": (sb * NH + h) % 2}
                for tp2 in range(NT2):
                    # tree emitted mid-loop: early C-drain DVE copies aren't
                    # stuck behind it, and it still beats the ones-matmul
                    if tp2 == 3 and prev is not None:
                        finish_prev_tree(prev)
                    ps = ps_mm.tile([128, W2], f32, tag="mm", name="ps_sc")
                    for half in range(2):
                        tb = tp2 * 2 + half
                        nc.tensor.matmul(
                            ps[:, half * SBW:(half + 1) * SBW],
                            lhsT=kt[h][:, tb * 128:(tb + 1) * 128],
                            rhs=qt[h][:, ssl], start=True, stop=True,
                            skip_group_check=True)
                    p2 = ppool.tile([128, W2], bf16, tag="p", name="ptile")
                    nc.scalar.activation(p2, ps, Exp, scale=SCALE)
                    cur["p2"].append(p2)
                    if prev is not None:
                        hp, sslp = prev["h"], prev["ssl"]
                        for half in range(2):
                            tb = tp2 * 2 + half
                            nc.tensor.matmul(
                                prev["ops"],
                                lhsT=vt[tb][:, hp * 128:(hp + 1) * 128],
                                rhs=prev["p2"][tp2][:, half * SBW:(half + 1) * SBW],
                                start=(tb == 0), stop=(tb == NTB - 1),
                                skip_group_check=True)
                    drain_c(C_PER_SLOT)
                if prev is not None:
                    finish_prev_post(prev)
                if h == 1 and sb >= 1:
                    cqueue.extend(make_c_work(sb - 1, [lambda: z_ps]))
                state = cur

            # epilogue: flush the last attention iteration + final out-proj
            prev = state
            finish_prev_tree(prev)
            par = prev["par"]
            prev["ops"] = ops2[:, par * SBW:(par + 1) * SBW]
            for tp2 in range(NT2):
                hp, sslp = prev["h"], prev["ssl"]
                for half in range(2):
                    tb = tp2 * 2 + half
                    nc.tensor.matmul(
                        prev["ops"], lhsT=vt[tb][:, hp * 128:(hp + 1) * 128],
                        rhs=prev["p2"][tp2][:, half * SBW:(half + 1) * SBW],
                        start=(tb == 0), stop=(tb == NTB - 1),
                        skip_group_check=True)
                drain_c(C_PER_SLOT)
            finish_prev_post(prev)
            drain_c(len(cqueue))
            # final s-block out-proj: alternate z bank with fresh ps_mm
            # tiles (sc is done) so back-to-back groups double-buffer
            def mm_ps():
                return ps_mm.tile([128, SBW], f32, tag="mm", name="ps_zf")

            for fn in make_c_work(NSB - 1, [lambda: z_ps, mm_ps]):
                fn()

    nc.compile()
    return nc


def _bf16(a):
    return np.ascontiguousarray(a).astype(ml_dtypes.bfloat16)


def _in_maps(inputs):
    q = np.asarray(inputs["query"], dtype=np.float32)
    k = np.asarray(inputs["key_in"], dtype=np.float32)
    v = np.asarray(inputs["value"], dtype=np.float32)
    Wq = np.asarray(inputs["Wq"], dtype=np.float32)
    Wk = np.asarray(inputs["Wk"], dtype=np.float32)
    Wv = np.asarray(inputs["Wv"], dtype=np.float32)
    Wo = np.asarray(inputs["Wo"], dtype=np.float32)
    bq = np.asarray(inputs["bq"], dtype=np.float32)
    bk = np.asarray(inputs["bk"], dtype=np.float32)

    xT = [[_bf16(x[b].T) for b in range(B)] for x in (q, k, v)]
    maps = []
    for c in range(NCORES):
        b, g = divmod(c, TP)
        sl = slice(g * CL, (g + 1) * CL)
        maps.append({
            "xqT": xT[0][b], "xkT": xT[1][b], "xvT": xT[2][b],
            "wqT": _bf16(Wq[sl, :].T), "wkT": _bf16(Wk[sl, :].T),
            "wvT": _bf16(Wv[sl, :].T), "woT": _bf16(Wo[:, sl].T),
            "bq": np.ascontiguousarray(bq[sl]),
            "bk": np.ascontiguousarray(bk[sl]),
        })
    return maps


TRACE = False
TMPDIR = None
LAST_RESULT = None


def kernel(**inputs):
    global _NC, LAST_RESULT
    from concourse.bass_utils import run_bass_kernel_spmd

    if _NC is None:
        _NC = _build_nc()
    maps = _in_maps(inputs)
    res = run_bass_kernel_spmd(_NC, maps, core_ids=list(range(NCORES)),
                               trace=TRACE, tmpdir=TMPDIR)
    LAST_RESULT = res

    Wo = np.asarray(inputs["Wo"], dtype=np.float32)
    bv = np.asarray(inputs["bv"], dtype=np.float32)
    bo = np.asarray(inputs["bo"], dtype=np.float32)
    out = np.zeros((B, S, D), dtype=np.float32)
    for c in range(NCORES):
        b, _ = divmod(c, TP)
        out[b] += res.results[c]["zT"].astype(np.float32).T
    out += (bo + Wo @ bv)[None, None, :]
    return out


# revision 19
# speedup vs baseline: 1.3757x; 1.0747x over previous
"""Trainium2 Bass kernel for CustomMultiHeadAttention.

Problem: B=2, S=2048, D=2048, H=16 heads of Hd=128; y = MHA(q, k, v) with
torch-Linear-style projections (x @ W.T + b) and out projection.

Sharding (8 NeuronCores): data-parallel over batch (2 groups) x tensor-
parallel over heads (4 heads = 512 channels per core). Each core computes
its 4 heads' Q/K/V projections, attention, and a column-sharded partial of
the out projection; the host sums the 4 partials per batch and adds
bo + Wo @ bv (bv commutes with the out projection, so it never goes
on-device).

Per-core device program (all matmuls bf16, K=128 contraction):
  Phase A (projections): Q^T/K^T channels-major [512c, S]; V sequence-major
    [S, 512e]. DMA order puts wq + the first x panel first so the PE starts
    ~15us in; wk/wv stream behind, wo is deferred to phase B.
  Phase B+C (attention + out-proj, software-pipelined): per (h, s-block)
    iteration the PE emits scores for iter i interleaved (per 2-t-block
    slot) with PV accumulation for iter i-1 and a trickle of out-proj
    matmuls for s-block sb-1.  Softmax denominators come from a DVE add
    chain over the 8 exp tiles + ONE ones-column matmul (512 cols) instead
    of 16 -- the PE streams P only once for PV.  1/l via
    reciprocal_approx_fast on the [1,512] row, then GPSIMD broadcast and
    one DVE mul.  Out-proj partials drain via DVE copy + DMA as they
    complete; only the last s-block's out-proj runs as a tail.
"""

import math

import numpy as np
import ml_dtypes

B = 2
S = 2048
D = 2048
HD = 128          # head dim
TP = 4            # head-group (tensor-parallel) factor
CL = D // TP      # 512 local channels = 4 heads per core
NCORES = 8

_NC = None


def _build_nc(s=S, d=D, cl=CL):
    """Build the per-core Bass program (SPMD: same program, 8 cores)."""
    from contextlib import ExitStack

    import concourse.bass as bass
    import concourse.mybir as mybir
    import concourse.tile as tile
    from concourse import bacc, bass_isa

    f32 = mybir.dt.float32
    bf16 = mybir.dt.bfloat16
    Exp = mybir.ActivationFunctionType.Exp

    SBW = 512                 # matmul moving free dim / PSUM bank width
    W2 = 2 * SBW              # two-bank psum tile width
    NSB = s // SBW            # s blocks
    NTB = s // 128            # t blocks (key/value seq chunks)
    NT2 = NTB // 2            # tp2 slots per (h, sb)
    KC = d // 128             # contraction chunks over model dim
    NH = cl // HD             # local heads
    SCALE = 1.0 / math.sqrt(HD)
    C_PER_SLOT = 3            # out-proj matmuls trickled per tp2 slot

    nc = bacc.Bacc("TRN2", target_bir_lowering=False, debug=False)

    xqT = nc.dram_tensor("xqT", [d, s], bf16, kind="ExternalInput").ap()
    xkT = nc.dram_tensor("xkT", [d, s], bf16, kind="ExternalInput").ap()
    xvT = nc.dram_tensor("xvT", [d, s], bf16, kind="ExternalInput").ap()
    wqT = nc.dram_tensor("wqT", [d, cl], bf16, kind="ExternalInput").ap()
    wkT = nc.dram_tensor("wkT", [d, cl], bf16, kind="ExternalInput").ap()
    wvT = nc.dram_tensor("wvT", [d, cl], bf16, kind="ExternalInput").ap()
    woT = nc.dram_tensor("woT", [cl, d], bf16, kind="ExternalInput").ap()
    bq = nc.dram_tensor("bq", [cl], f32, kind="ExternalInput").ap()
    bk = nc.dram_tensor("bk", [cl], f32, kind="ExternalInput").ap()
    zT = nc.dram_tensor("zT", [d, s], bf16, kind="ExternalOutput").ap()

    with tile.TileContext(nc) as tc, ExitStack() as ctx:
        const = ctx.enter_context(tc.tile_pool(name="const", bufs=1))
        qkv = ctx.enter_context(tc.tile_pool(name="qkv", bufs=1))
        wop = ctx.enter_context(tc.tile_pool(name="wop", bufs=1))
        # PSUM budget (8 banks, pools are 2-bank granular): sc 2x2 + one
        # 2-bank tile whose halves ping-pong as PV accumulators + one
        # 2-bank tile split into out-proj group bank | l-row bank.
        ps_mm = ctx.enter_context(tc.tile_pool(name="ps_mm", bufs=2, space="PSUM"))
        ps_ot = ctx.enter_context(tc.tile_pool(name="ps_ot", bufs=1, space="PSUM"))
        ps_zl = ctx.enter_context(tc.tile_pool(name="ps_zl", bufs=1, space="PSUM"))

        ones_col = const.tile([128, 1], bf16, tag="ones_col")
        nc.vector.memset(ones_col, 1.0)

        # Biases land via DMA into *_dma, then hop to *_sb on the DVE so
        # downstream tensor_scalar ops only carry their PE wait.
        def load_bias(b_dram, nm):
            b_dma = const.tile([128, NH], f32, tag=f"{nm}d", name=f"{nm}d")
            nc.sync.dma_start(b_dma, b_dram.rearrange("(m p) -> p m", p=128))
            b_sb = const.tile([128, NH], f32, tag=nm, name=nm)
            nc.vector.tensor_copy(b_sb, b_dma)
            return b_sb

        bq_sb = load_bias(bq, "bq")
        bk_sb = load_bias(bk, "bk")

        # wo tile allocated now, DMA deferred until phase B (saves startup
        # DMA bandwidth for wq + the first x panel).
        wo_sb = wop.tile([128, NH, d], bf16, tag="wo")

        # persistent activation tiles
        qt = [qkv.tile([128, s], bf16, tag=f"qt{h}", name=f"qt{h}") for h in range(NH)]
        kt = [qkv.tile([128, s], bf16, tag=f"kt{h}", name=f"kt{h}") for h in range(NH)]
        vt = [qkv.tile([128, cl], bf16, tag=f"vt{t}", name=f"vt{t}") for t in range(NTB)]
        ot = [qkv.tile([128, s], bf16, tag=f"ot{h}", name=f"ot{h}") for h in range(NH)]

        # ---- Phase A: projections -------------------------------------
        with tc.tile_pool(name="wqkv", bufs=1) as wp, \
             tc.tile_pool(name="panels", bufs=2) as panels:

            wq_sb = wp.tile([128, KC, cl], bf16, tag="wq")
            wq_re = wqT.rearrange("(k p) m -> p k m", p=128)
            # split across two queues so the gating transfer finishes sooner
            nc.sync.dma_start(wq_sb[:, 0:KC // 2], wq_re[:, 0:KC // 2])
            nc.sync.dma_start(wq_sb[:, KC // 2:KC], wq_re[:, KC // 2:KC])
            wk_sb = wp.tile([128, KC, cl], bf16, tag="wk")
            wv_sb = wp.tile([128, KC, cl], bf16, tag="wv")

            def load_panel(x_dram, lo, hi, nm, split):
                xp = panels.tile([128, KC, hi - lo], bf16, tag="xpanel", name=nm)
                xr = x_dram[:, lo:hi].rearrange("(k p) s -> p k s", p=128)
                if split:
                    nc.sync.dma_start(xp[:, 0:KC // 2], xr[:, 0:KC // 2])
                    nc.sync.dma_start(xp[:, KC // 2:KC], xr[:, KC // 2:KC])
                else:
                    nc.sync.dma_start(xp, xr)
                return xp

            def qk_proj(x_dram, w_sb, b_sb, outs, hook=None):
                # outs[m][c, s] = sum_d W[m*128+c, d] x[s, d]  (+ bias)
                for np_ in range(NSB // 2):
                    xp = load_panel(x_dram, np_ * W2, (np_ + 1) * W2,
                                    f"xp{np_}", split=(np_ == 0 and hook))
                    if np_ == 0 and hook:
                        hook()
                    for m in range(NH):
                        ps = ps_mm.tile([128, W2], f32, tag="mm", name="ps_proj")
                        for k in range(KC):
                            w_ap = w_sb[:, k, m * 128:(m + 1) * 128]
                            nc.tensor.matmul(
                                ps[:, 0:SBW], lhsT=w_ap, rhs=xp[:, k, 0:SBW],
                                start=(k == 0), stop=(k == KC - 1))
                            nc.tensor.matmul(
                                ps[:, SBW:W2], lhsT=w_ap, rhs=xp[:, k, SBW:W2],
                                start=(k == 0), stop=(k == KC - 1))
                        nc.vector.tensor_scalar_add(
                            outs[m][:, np_ * W2:(np_ + 1) * W2], ps,
                            b_sb[:, m:m + 1])

            def issue_wk_wv():
                nc.sync.dma_start(wk_sb, wkT.rearrange("(k p) m -> p k m", p=128))
                nc.sync.dma_start(wv_sb, wvT.rearrange("(k p) m -> p k m", p=128))

            qk_proj(xqT, wq_sb, bq_sb, qt, hook=issue_wk_wv)
            qk_proj(xkT, wk_sb, bk_sb, kt)

            # V sequence-major: vt[t][tt, e] = sum_d x[t*128+tt, d] Wv[e, d]
            for n in range(NSB):
                xp = load_panel(xvT, n * SBW, (n + 1) * SBW, f"xpv{n}", False)
                for tsub in range(SBW // 128):
                    t = n * (SBW // 128) + tsub
                    ps = ps_mm.tile([128, cl], f32, tag="mm", name="ps_v")
                    for k in range(KC):
                        nc.tensor.matmul(
                            ps, lhsT=xp[:, k, tsub * 128:(tsub + 1) * 128],
                            rhs=wv_sb[:, k, :],
                            start=(k == 0), stop=(k == KC - 1))
                    nc.vector.tensor_copy(vt[t], ps)

        # ---- Phase B+C: attention + out-proj, software-pipelined -------
        with tc.tile_pool(name="ptiles", bufs=2 * NT2) as ppool, \
             tc.tile_pool(name="accp", bufs=10) as accp, \
             tc.tile_pool(name="small", bufs=4) as small, \
             tc.tile_pool(name="zout", bufs=3) as zpool:

            nc.sync.dma_start(wo_sb, woT.rearrange("(k p) m -> p k m", p=128))

            # persistent psum: PV accumulator halves ping-pong per
            # iteration; z bank | l row share the other 2-bank tile
            ops2 = ps_ot.tile([128, W2], f32, tag="ops2", name="ops2")
            pz2 = ps_zl.tile([128, W2], f32, tag="pz2", name="pz2")
            z_ps = pz2[:, 0:SBW]
            lps = pz2[0:1, SBW:W2]

            # --- out-proj (phase C) work queue: per-matmul closures ---
            cqueue = []

            def make_c_work(sb, ps_get):
                ssl = slice(sb * SBW, (sb + 1) * SBW)
                box = {}

                def mk(dd, eb, getter):
                    def emit():
                        if eb == 0:
                            box["ps"] = getter()
                        nc.tensor.matmul(
                            box["ps"], lhsT=wo_sb[:, eb, dd * 128:(dd + 1) * 128],
                            rhs=ot[eb][:, ssl],
                            start=(eb == 0), stop=(eb == NH - 1),
                            skip_group_check=True)
                        if eb == NH - 1:
                            zt = zpool.tile([128, SBW], bf16, tag="z", name="z_sb")
                            nc.vector.tensor_copy(zt, box["ps"])
                            nc.sync.dma_start(
                                zT[dd * 128:(dd + 1) * 128, ssl], zt)
                    return emit

                return [mk(dd, eb, ps_get[dd % len(ps_get)])
                        for dd in range(KC) for eb in range(NH)]

            def drain_c(k):
                for _ in range(min(k, len(cqueue))):
                    cqueue.pop(0)()

            # --- per-iteration state for the software pipeline ---
            def finish_prev_tree(st):
                # DVE: binary-tree sum of the 8 exp tiles in bf16 (2x DVE
                # rate; ~0.4% worst-case on l, well inside tolerance).
                t = [accp.tile([128, W2], bf16, tag="acc", name=f"acc{j}")
                     for j in range(4)]
                for j in range(4):
                    nc.vector.tensor_add(t[j], st["p2"][2 * j], st["p2"][2 * j + 1])
                nc.vector.tensor_add(t[0], t[0], t[1])
                nc.vector.tensor_add(t[2], t[2], t[3])
                nc.vector.tensor_add(t[0], t[0], t[2])
                acc2 = accp.tile([128, SBW], bf16, tag="acc2", name="acc2")
                nc.vector.tensor_add(acc2, t[0][:, 0:SBW], t[0][:, SBW:W2])
                st["acc2"] = acc2

            def finish_prev_post(st):
                # PE: single ones-matmul for the softmax denominator row
                nc.tensor.matmul(lps, lhsT=ones_col, rhs=st["acc2"],
                                 start=True, stop=True, skip_group_check=True)
                # DVE: 1/l on the row, GPSIMD broadcast
                rl = small.tile([1, SBW], f32, tag="rl", name="rl")
                nc.vector.reciprocal_approx_fast(out=rl, in_=lps)
                rb = small.tile([128, SBW], f32, tag="rb", name="rb")
                nc.gpsimd.partition_broadcast(rb, rl)
                st["rb"] = rb

            def finish_prev_mul(st):
                h, ssl = st["h"], st["ssl"]
                nc.vector.tensor_mul(ot[h][:, ssl], st["ops"], st["rb"])

            state = None
            iters = [(sb, h) for sb in range(NSB) for h in range(NH)]
            for sb, h in iters:
                ssl = slice(sb * SBW, (sb + 1) * SBW)
                prev = state
                if prev is not None:
                    par = prev["par"]
                    prev["ops"] = ops2[:, par * SBW:(par + 1) * SBW]
                cur = {"h": h, "sb": sb, "ssl": ssl, "p2": [],
                       "par": (sb * NH + h) % 2}
                for tp2 in range(NT2):
                    # pv/C emitted BEFORE sc: while sc waits on the exp
                    # ping-pong the PE still has dependency-free work
                    if prev is not None:
                        hp = prev["h"]
                        for half in range(2):
                            tb = tp2 * 2 + half
                            nc.tensor.matmul(
                                prev["ops"],
                                lhsT=vt[tb][:, hp * 128:(hp + 1) * 128],
                                rhs=prev["p2"][tp2][:, half * SBW:(half + 1) * SBW],
                                start=(tb == 0), stop=(tb == NTB - 1),
                                skip_group_check=True)
                    drain_c(C_PER_SLOT)
                    if tp2 == 1 and prev is not None:
                        finish_prev_tree(prev)
                    ps = ps_mm.tile([128, W2], f32, tag="mm", name="ps_sc")
                    for half in range(2):
                        tb = tp2 * 2 + half
                        nc.tensor.matmul(
                            ps[:, half * SBW:(half + 1) * SBW],
                            lhsT=kt[h][:, tb * 128:(tb + 1) * 128],
                            rhs=qt[h][:, ssl], start=True, stop=True,
                            skip_group_check=True)
                    p2 = ppool.tile([128, W2], bf16, tag="p", name="ptile")
                    nc.scalar.activation(p2, ps, Exp, scale=SCALE)
                    cur["p2"].append(p2)
                    # normalize chain mid-iteration so mul(prev) lands
                    # before the next iteration's pv group-start
                    if tp2 == 5 and prev is not None:
                        finish_prev_post(prev)
                if prev is not None:
                    finish_prev_mul(prev)
                if h == 1 and sb >= 1:
                    cqueue.extend(make_c_work(sb - 1, [lambda: z_ps]))
                state = cur

            # epilogue: flush the last attention iteration + final out-proj
            prev = state
            finish_prev_tree(prev)
            par = prev["par"]
            prev["ops"] = ops2[:, par * SBW:(par + 1) * SBW]
            for tp2 in range(NT2):
                hp, sslp = prev["h"], prev["ssl"]
                for half in range(2):
                    tb = tp2 * 2 + half
                    nc.tensor.matmul(
                        prev["ops"], lhsT=vt[tb][:, hp * 128:(hp + 1) * 128],
                        rhs=prev["p2"][tp2][:, half * SBW:(half + 1) * SBW],
                        start=(tb == 0), stop=(tb == NTB - 1),
                        skip_group_check=True)
                drain_c(C_PER_SLOT)
            finish_prev_post(prev)
            finish_prev_mul(prev)
            drain_c(len(cqueue))
            # final s-block out-proj: alternate z bank with fresh ps_mm
            # tiles (sc is done) so back-to-back groups double-buffer
            def mm_ps():
                return ps_mm.tile([128, SBW], f32, tag="mm", name="ps_zf")

            for fn in make_c_work(NSB - 1, [lambda: z_ps, mm_ps]):
                fn()

    nc.compile()
    return nc


def _bf16(a):
    return np.ascontiguousarray(a).astype(ml_dtypes.bfloat16)


def _in_maps(inputs):
    q = np.asarray(inputs["query"], dtype=np.float32)
    k = np.asarray(inputs["key_in"], dtype=np.float32)
    v = np.asarray(inputs["value"], dtype=np.float32)
    Wq = np.asarray(inputs["Wq"], dtype=np.float32)
    Wk = np.asarray(inputs["Wk"], dtype=np.float32)
    Wv = np.asarray(inputs["Wv"], dtype=np.float32)
    Wo = np.asarray(inputs["Wo"], dtype=np.float32)
    bq = np.asarray(inputs["bq"], dtype=np.float32)
    bk = np.asarray(inputs["bk"], dtype=np.float32)

    xT = [[_bf16(x[b].T) for b in range(B)] for x in (q, k, v)]
    maps = []
    for c in range(NCORES):
        b, g = divmod(c, TP)
        sl = slice(g * CL, (g + 1) * CL)
        maps.append({
            "xqT": xT[0][b], "xkT": xT[1][b], "xvT": xT[2][b],
            "wqT": _bf16(Wq[sl, :].T), "wkT": _bf16(Wk[sl, :].T),
            "wvT": _bf16(Wv[sl, :].T), "woT": _bf16(Wo[:, sl].T),
            "bq": np.ascontiguousarray(bq[sl]),
            "bk": np.ascontiguousarray(bk[sl]),
        })
    return maps


TRACE = False
TMPDIR = None
LAST_RESULT = None


def kernel(**inputs):
    global _NC, LAST_RESULT
    from concourse.bass_utils import run_bass_kernel_spmd

    if _NC is None:
        _NC = _build_nc()
    maps = _in_maps(inputs)
    res = run_bass_kernel_spmd(_NC, maps, core_ids=list(range(NCORES)),
                               trace=TRACE, tmpdir=TMPDIR)
    LAST_RESULT = res

    Wo = np.asarray(inputs["Wo"], dtype=np.float32)
    bv = np.asarray(inputs["bv"], dtype=np.float32)
    bo = np.asarray(inputs["bo"], dtype=np.float32)
    out = np.zeros((B, S, D), dtype=np.float32)
    for c in range(NCORES):
        b, _ = divmod(c, TP)
        out[b] += res.results[c]["zT"].astype(np.float32).T
    out += (bo + Wo @ bv)[None, None, :]
    return out
